# revision 4
# baseline (speedup 1.0000x reference)
"""Causal multi-head attention block (QKV proj -> causal attention -> out proj)
for Trainium2, distributed over 8 NeuronCores.

Sharding: core c handles batch b = c//2 and head-group g = c%2 (8 of 16 heads).
Each core computes qkv for its group's columns of w_attn, runs causal attention
for its 8 heads, and multiplies by its group's rows of w_proj, producing a
partial y[b]. The host sums the two partials per batch and adds b_proj.

All device matmuls run in fp32r (full-rate fp32 streaming mode). The kernel
works in transposed layouts end-to-end (host passes x[b].T, device returns
y[b].T) so no on-device transposes are needed:
  q^T,k^T = w_{q,k}^T-chunks @ x^T      [cols, tok]
  s^T     = k_h^T-chunks    @ q_h^T     [k_tok, q_tok]  (exp+mask -> p^T)
  out^T   = [v_h | 1]^T     @ p^T       [65, q_tok]     (row 64 = softmax sums)
  y^T     = w_proj-chunks   @ out_norm^T
"""

import math
import sys

import numpy as np

if "/opt/trn_rl_repo" not in sys.path:
    sys.path.insert(0, "/opt/trn_rl_repo")

B, S, D = 4, 1024, 1024
H = 16
HPG = 8              # heads per group (2 groups of 8)
hd = D // H          # 64
GC = HPG * hd        # 512 cols per group for each of q,k,v
P = 128
DC = D // P          # 8 contraction chunks
NEG = None           # masking is multiplicative (exact zero), not additive

_CACHE = {}


def _build():
    import concourse.mybir as mybir
    import concourse.tile as tile
    from concourse import bacc
    from concourse.masks import make_upper_triangular

    f32 = mybir.dt.float32
    f32r = mybir.dt.float32r
    Exp = mybir.ActivationFunctionType.Exp
    mult = mybir.AluOpType.mult

    nc = bacc.Bacc("TRN2", target_bir_lowering=False, debug=False, num_devices=8)
    xT = nc.dram_tensor("xT", [D, S], f32r, kind="ExternalInput").ap()
    wq = nc.dram_tensor("wq", [D, GC], f32r, kind="ExternalInput").ap()
    wk = nc.dram_tensor("wk", [D, GC], f32r, kind="ExternalInput").ap()
    wv = nc.dram_tensor("wv", [D, GC], f32r, kind="ExternalInput").ap()
    wp = nc.dram_tensor("wp", [GC, D], f32r, kind="ExternalInput").ap()
    yT = nc.dram_tensor("yT", [D, S], f32, kind="ExternalOutput").ap()

    scale = 1.0 / math.sqrt(hd)

    with tile.TileContext(nc) as tc:
        with tc.tile_pool(name="const", bufs=1) as const, \
             tc.tile_pool(name="big", bufs=1) as big, \
             tc.tile_pool(name="pt", bufs=6) as ptp, \
             tc.tile_pool(name="small", bufs=4) as small, \
             tc.tile_pool(name="yt", bufs=3) as ytp, \
             tc.tile_pool(name="ps", bufs=8, space="PSUM") as ps:

            tri = const.tile([P, P], f32, tag="tri")      # keep iff k_local <= q_local
            make_upper_triangular(nc, tri[:], val=1.0, diag=True)

            xt = big.tile([P, DC, S], f32r, tag="xt")
            nc.sync.dma_start(xt[:], xT.rearrange("(dc p) t -> p dc t", p=P))
            wqt = big.tile([P, DC, GC], f32r, tag="wq")
            nc.sync.dma_start(wqt[:], wq.rearrange("(dc p) c -> p dc c", p=P))
            wkt = big.tile([P, DC, GC], f32r, tag="wk")
            nc.sync.dma_start(wkt[:], wk.rearrange("(dc p) c -> p dc c", p=P))
            wvt = big.tile([P, DC, GC], f32r, tag="wv")
            nc.sync.dma_start(wvt[:], wv.rearrange("(dc p) c -> p dc c", p=P))
            wpt = big.tile([P, GC // P, D], f32r, tag="wp")
            nc.sync.dma_start(wpt[:], wp.rearrange("(cc p) o -> p cc o", p=P))

            # q^T/k^T for the group: [col(128), chunk, tok]; chunks 0-3 = q, 4-7 = k
            qkt = big.tile([P, 2 * GC // P, S], f32r, tag="qkt")
            # v padded with a ones column per head: [tok(128), tok_chunk, head, hd+1]
            vaug = big.tile([P, S // P, HPG, hd + 1], f32r, tag="vaug")
            ones64 = const.tile([P, S // P, HPG, 1], f32, tag="ones64")
            nc.any.memset(ones64[:], 1.0)
            nc.vector.tensor_copy(out=vaug[:, :, :, hd:hd + 1], in_=ones64[:])
            # normalized attention output ^T: [chan(128), chan_chunk, tok]
            outt = big.tile([P, GC // P, S], f32r, tag="outt")

            # ---- qkv projections ----
            for cc8 in range(8):                       # 4 q col-chunks then 4 k
                src = wqt if cc8 < 4 else wkt
                cbase = (cc8 % 4) * P
                for t5 in range(S // 512):
                    acc = ps.tile([P, 512], f32, tag="ps")
                    for dc in range(DC):
                        nc.tensor.matmul(
                            acc[:],
                            src[:, dc, cbase:cbase + P],
                            xt[:, dc, t5 * 512:(t5 + 1) * 512],
                            start=(dc == 0), stop=(dc == DC - 1),
                        )
                    nc.vector.tensor_copy(
                        out=qkt[:, cc8, t5 * 512:(t5 + 1) * 512], in_=acc[:])

            for t8 in range(S // P):
                acc = ps.tile([P, 512], f32, tag="ps")
                for dc in range(DC):
                    nc.tensor.matmul(
                        acc[:],
                        xt[:, dc, t8 * P:(t8 + 1) * P],
                        wvt[:, dc, :],
                        start=(dc == 0), stop=(dc == DC - 1),
                    )
                nc.vector.tensor_copy(
                    out=vaug[:, t8, :, 0:hd],
                    in_=acc[:].rearrange("p (h j) -> p h j", h=HPG))

            # ---- causal attention, transposed layouts ----
            for qc in range(S // 512):                 # 512-wide q chunks
                for h in range(HPG):
                    prow = 64 * (h % 2)
                    qh = qkt[prow:prow + hd, h // 2, :]
                    kh = qkt[prow:prow + hd, 4 + h // 2, :]
                    acc = ps.tile([P, 512], f32, tag="ps")
                    # k-blocks: diagonal p=0 first (full width, start=True),
                    # then fully-below blocks, then partial-width diagonals
                    order = [4 * qc] + list(range(4 * qc)) + \
                        [4 * qc + p for p in (1, 2, 3)]
                    for i, kb in enumerate(order):
                        p_off = kb - 4 * qc            # >=0 on diagonal blocks
                        start_col = max(0, p_off) * P
                        width = 512 - start_col
                        sp = ps.tile([P, 512], f32, tag="ps")
                        nc.tensor.matmul(
                            sp[:, :width],
                            kh[:, kb * P:(kb + 1) * P],
                            qh[:, qc * 512 + start_col:(qc + 1) * 512],
                            start=True, stop=True,
                        )
                        pt = ptp.tile([P, 512], f32r, tag="pt")
                        nc.scalar.activation(pt[:, :width], sp[:, :width],
                                             Exp, scale=scale)
                        if p_off >= 0:                 # triangular part of mask
                            nc.vector.tensor_tensor(
                                pt[:, 0:P], pt[:, 0:P], tri[:], mult)
                        nc.tensor.matmul(
                            acc[:hd + 1, start_col:512],
                            vaug[:, kb, h, :],
                            pt[:, :width],
                            start=(i == 0), stop=(i == len(order) - 1),
                        )
                    # normalize: out^T[:, q] /= rowsum[q] (psum row 64)
                    rcp = small.tile([1, 512], f32, tag="rcp")
                    with nc.allow_low_precision(reason="softmax denom recip"):
                        nc.vector.reciprocal(rcp[:], acc[hd:hd + 1, :])
                    bc = small.tile([hd, 512], f32, tag="bc")
                    nc.gpsimd.partition_broadcast(bc[:], rcp[:])
                    nc.vector.tensor_tensor(
                        outt[prow:prow + hd, h // 2, qc * 512:(qc + 1) * 512],
                        acc[0:hd, :], bc[:], mult)

            # ---- output projection: y^T = w_proj^T-chunks @ out^T ----
            for t5 in range(S // 512):
                for oc in range(D // P):
                    acc = ps.tile([P, 512], f32, tag="ps")
                    for cc in range(GC // P):
                        nc.tensor.matmul(
                            acc[:],
                            wpt[:, cc, oc * P:(oc + 1) * P],
                            outt[:, cc, t5 * 512:(t5 + 1) * 512],
                            start=(cc == 0), stop=(cc == GC // P - 1),
                        )
                    yt = ytp.tile([P, 512], f32, tag="yt")
                    nc.vector.tensor_copy(out=yt[:], in_=acc[:])
                    nc.sync.dma_start(
                        yT[oc * P:(oc + 1) * P, t5 * 512:(t5 + 1) * 512], yt[:])

    nc.compile()
    return nc


def _get_nc():
    if "nc" not in _CACHE:
        _CACHE["nc"] = _build()
    return _CACHE["nc"]


def make_in_maps(x, w_attn):
    """Per-core input shards (core c -> batch c//2, head-group c%2)."""
    in_maps = []
    xTs = [np.ascontiguousarray(x[b].T) for b in range(B)]
    for c in range(8):
        b, g = divmod(c, 2)
        in_maps.append({
            "xT": xTs[b],
            "wq": np.ascontiguousarray(w_attn[:, g * GC:(g + 1) * GC]),
            "wk": np.ascontiguousarray(w_attn[:, D + g * GC:D + (g + 1) * GC]),
            "wv": np.ascontiguousarray(w_attn[:, 2 * D + g * GC:2 * D + (g + 1) * GC]),
            "wp": None,  # filled below
        })
    return in_maps


def kernel(x, w_attn, b_attn, w_proj, b_proj):
    x = np.asarray(x, dtype=np.float32)
    w_attn = np.asarray(w_attn, dtype=np.float32)
    b_attn = np.asarray(b_attn, dtype=np.float32)
    w_proj = np.asarray(w_proj, dtype=np.float32)
    b_proj = np.asarray(b_proj, dtype=np.float32)

    if np.any(b_attn):
        # Spec guarantees b_attn == 0 (fill: zeros); exact fallback if not.
        return _numpy_reference(x, w_attn, b_attn, w_proj, b_proj)

    from concourse.bass_utils import run_bass_kernel_spmd

    nc = _get_nc()
    in_maps = make_in_maps(x, w_attn)
    for c in range(8):
        g = c % 2
        in_maps[c]["wp"] = np.ascontiguousarray(w_proj[g * GC:(g + 1) * GC, :])

    res = run_bass_kernel_spmd(nc, in_maps, core_ids=list(range(8)))
    y = np.empty((B, S, D), np.float32)
    for b in range(B):
        y[b] = res.results[2 * b]["yT"].T + res.results[2 * b + 1]["yT"].T + b_proj
    return y


def _numpy_reference(x, w_attn, b_attn, w_proj, b_proj):
    qkv = x @ w_attn + b_attn
    q, k, v = np.split(qkv, 3, axis=-1)

    def heads(t):
        return t.reshape(B, S, H, hd).transpose(0, 2, 1, 3)

    q, k, v = heads(q), heads(k), heads(v)
    scores = np.einsum("bhqd,bhkd->bhqk", q, k) / np.sqrt(np.float32(hd))
    causal = np.tril(np.ones((S, S), dtype=bool))[None, None]
    scores = np.where(causal, scores, -1e9)
    scores -= scores.max(axis=-1, keepdims=True)
    attn = np.exp(scores)
    attn /= attn.sum(axis=-1, keepdims=True)
    out = np.einsum("bhqk,bhkd->bhqd", attn, v)
    out = out.transpose(0, 2, 1, 3).reshape(B, S, D)
    return out @ w_proj + b_proj


# revision 5
# speedup vs baseline: 19.1148x; 19.1148x over previous
"""Causal multi-head attention block (QKV proj -> causal attention -> out proj)
for Trainium2, distributed over 8 NeuronCores.

Sharding: core c handles batch b = c//2 and head-group g = c%2 (8 of 16 heads).
Each core computes qkv for its group's columns of w_attn, runs causal attention
for its 8 heads, and multiplies by its group's rows of w_proj, producing a
partial y[b]. The host sums the two partials per batch and adds b_proj.

All device matmuls run in fp32r (full-rate fp32 streaming mode). The kernel
works in transposed layouts end-to-end (host passes x[b].T, device returns
y[b].T) so no on-device transposes are needed:
  q^T,k^T = w_{q,k}^T-chunks @ x^T      [cols, tok]
  s^T     = k_h^T-chunks    @ q_h^T     [k_tok, q_tok]  (exp+mask -> p^T)
  out^T   = [v_h | 1]^T     @ p^T       [65, q_tok]     (row 64 = softmax sums)
  y^T     = w_proj-chunks   @ out_norm^T
"""

import math
import sys

import numpy as np

if "/opt/trn_rl_repo" not in sys.path:
    sys.path.insert(0, "/opt/trn_rl_repo")

B, S, D = 4, 1024, 1024
H = 16
HPG = 8              # heads per group (2 groups of 8)
hd = D // H          # 64
GC = HPG * hd        # 512 cols per group for each of q,k,v
P = 128
DC = D // P          # 8 contraction chunks
NEG = None           # masking is multiplicative (exact zero), not additive

_CACHE = {}


def _build(repeat=1):
    import concourse.mybir as mybir
    import concourse.tile as tile
    from concourse import bacc
    from concourse.masks import make_upper_triangular

    f32 = mybir.dt.float32
    f32r = mybir.dt.float32r
    Exp = mybir.ActivationFunctionType.Exp
    mult = mybir.AluOpType.mult

    nc = bacc.Bacc("TRN2", target_bir_lowering=False, debug=False, num_devices=8)
    xT = nc.dram_tensor("xT", [D, S], f32r, kind="ExternalInput").ap()
    wq = nc.dram_tensor("wq", [D, GC], f32r, kind="ExternalInput").ap()
    wk = nc.dram_tensor("wk", [D, GC], f32r, kind="ExternalInput").ap()
    wv = nc.dram_tensor("wv", [D, GC], f32r, kind="ExternalInput").ap()
    wp = nc.dram_tensor("wp", [GC, D], f32r, kind="ExternalInput").ap()
    yT = nc.dram_tensor("yT", [D, S], f32, kind="ExternalOutput").ap()

    scale = 1.0 / math.sqrt(hd)

    with tile.TileContext(nc) as tc:
        with tc.tile_pool(name="const", bufs=1) as const, \
             tc.tile_pool(name="big", bufs=1) as big, \
             tc.tile_pool(name="pt", bufs=6) as ptp, \
             tc.tile_pool(name="small", bufs=4) as small, \
             tc.tile_pool(name="yt", bufs=3) as ytp, \
             tc.tile_pool(name="ps", bufs=8, space="PSUM") as ps:

          for _rep in range(repeat):
            tri = const.tile([P, P], f32, tag="tri")      # keep iff k_local <= q_local
            make_upper_triangular(nc, tri[:], val=1.0, diag=True)

            xt = big.tile([P, DC, S], f32r, tag="xt")
            nc.sync.dma_start(xt[:], xT.rearrange("(dc p) t -> p dc t", p=P))
            wqt = big.tile([P, DC, GC], f32r, tag="wq")
            nc.sync.dma_start(wqt[:], wq.rearrange("(dc p) c -> p dc c", p=P))
            wkt = big.tile([P, DC, GC], f32r, tag="wk")
            nc.sync.dma_start(wkt[:], wk.rearrange("(dc p) c -> p dc c", p=P))
            wvt = big.tile([P, DC, GC], f32r, tag="wv")
            nc.sync.dma_start(wvt[:], wv.rearrange("(dc p) c -> p dc c", p=P))
            wpt = big.tile([P, GC // P, D], f32r, tag="wp")
            nc.sync.dma_start(wpt[:], wp.rearrange("(cc p) o -> p cc o", p=P))

            # q^T/k^T for the group: [col(128), chunk, tok]; chunks 0-3 = q, 4-7 = k
            qkt = big.tile([P, 2 * GC // P, S], f32r, tag="qkt")
            # v padded with a ones column per head: [tok(128), tok_chunk, head, hd+1]
            vaug = big.tile([P, S // P, HPG, hd + 1], f32r, tag="vaug")
            ones64 = const.tile([P, S // P, HPG, 1], f32, tag="ones64")
            nc.any.memset(ones64[:], 1.0)
            nc.vector.tensor_copy(out=vaug[:, :, :, hd:hd + 1], in_=ones64[:])
            # normalized attention output ^T: [chan(128), chan_chunk, tok]
            outt = big.tile([P, GC // P, S], f32r, tag="outt")

            # ---- qkv projections ----
            for cc8 in range(8):                       # 4 q col-chunks then 4 k
                src = wqt if cc8 < 4 else wkt
                cbase = (cc8 % 4) * P
                for t5 in range(S // 512):
                    acc = ps.tile([P, 512], f32, tag="ps")
                    for dc in range(DC):
                        nc.tensor.matmul(
                            acc[:],
                            src[:, dc, cbase:cbase + P],
                            xt[:, dc, t5 * 512:(t5 + 1) * 512],
                            start=(dc == 0), stop=(dc == DC - 1),
                        )
                    nc.vector.tensor_copy(
                        out=qkt[:, cc8, t5 * 512:(t5 + 1) * 512], in_=acc[:])

            for t8 in range(S // P):
                acc = ps.tile([P, 512], f32, tag="ps")
                for dc in range(DC):
                    nc.tensor.matmul(
                        acc[:],
                        xt[:, dc, t8 * P:(t8 + 1) * P],
                        wvt[:, dc, :],
                        start=(dc == 0), stop=(dc == DC - 1),
                    )
                nc.vector.tensor_copy(
                    out=vaug[:, t8, :, 0:hd],
                    in_=acc[:].rearrange("p (h j) -> p h j", h=HPG))

            # ---- causal attention, transposed layouts ----
            for qc in range(S // 512):                 # 512-wide q chunks
                for h in range(HPG):
                    prow = 64 * (h % 2)
                    qh = qkt[prow:prow + hd, h // 2, :]
                    kh = qkt[prow:prow + hd, 4 + h // 2, :]
                    acc = ps.tile([P, 512], f32, tag="ps")
                    # k-blocks: diagonal p=0 first (full width, start=True),
                    # then fully-below blocks, then partial-width diagonals
                    order = [4 * qc] + list(range(4 * qc)) + \
                        [4 * qc + p for p in (1, 2, 3)]
                    for i, kb in enumerate(order):
                        p_off = kb - 4 * qc            # >=0 on diagonal blocks
                        start_col = max(0, p_off) * P
                        width = 512 - start_col
                        sp = ps.tile([P, 512], f32, tag="ps")
                        nc.tensor.matmul(
                            sp[:, :width],
                            kh[:, kb * P:(kb + 1) * P],
                            qh[:, qc * 512 + start_col:(qc + 1) * 512],
                            start=True, stop=True,
                        )
                        pt = ptp.tile([P, 512], f32r, tag="pt")
                        nc.scalar.activation(pt[:, :width], sp[:, :width],
                                             Exp, scale=scale)
                        if p_off >= 0:                 # triangular part of mask
                            nc.vector.tensor_tensor(
                                pt[:, 0:P], pt[:, 0:P], tri[:], mult)
                        nc.tensor.matmul(
                            acc[:hd + 1, start_col:512],
                            vaug[:, kb, h, :],
                            pt[:, :width],
                            start=(i == 0), stop=(i == len(order) - 1),
                        )
                    # normalize: out^T[:, q] /= rowsum[q] (psum row 64)
                    rcp = small.tile([1, 512], f32, tag="rcp")
                    with nc.allow_low_precision(reason="softmax denom recip"):
                        nc.vector.reciprocal(rcp[:], acc[hd:hd + 1, :])
                    bc = small.tile([hd, 512], f32, tag="bc")
                    nc.gpsimd.partition_broadcast(bc[:], rcp[:])
                    nc.vector.tensor_tensor(
                        outt[prow:prow + hd, h // 2, qc * 512:(qc + 1) * 512],
                        acc[0:hd, :], bc[:], mult)

            # ---- output projection: y^T = w_proj^T-chunks @ out^T ----
            for t5 in range(S // 512):
                for oc in range(D // P):
                    acc = ps.tile([P, 512], f32, tag="ps")
                    for cc in range(GC // P):
                        nc.tensor.matmul(
                            acc[:],
                            wpt[:, cc, oc * P:(oc + 1) * P],
                            outt[:, cc, t5 * 512:(t5 + 1) * 512],
                            start=(cc == 0), stop=(cc == GC // P - 1),
                        )
                    yt = ytp.tile([P, 512], f32, tag="yt")
                    nc.vector.tensor_copy(out=yt[:], in_=acc[:])
                    nc.sync.dma_start(
                        yT[oc * P:(oc + 1) * P, t5 * 512:(t5 + 1) * 512], yt[:])

    nc.compile()
    return nc


def _get_nc(repeat=1):
    key = ("nc", repeat)
    if key not in _CACHE:
        _CACHE[key] = _build(repeat)
    return _CACHE[key]


def make_in_maps(x, w_attn):
    """Per-core input shards (core c -> batch c//2, head-group c%2)."""
    in_maps = []
    xTs = [np.ascontiguousarray(x[b].T) for b in range(B)]
    for c in range(8):
        b, g = divmod(c, 2)
        in_maps.append({
            "xT": xTs[b],
            "wq": np.ascontiguousarray(w_attn[:, g * GC:(g + 1) * GC]),
            "wk": np.ascontiguousarray(w_attn[:, D + g * GC:D + (g + 1) * GC]),
            "wv": np.ascontiguousarray(w_attn[:, 2 * D + g * GC:2 * D + (g + 1) * GC]),
            "wp": None,  # filled below
        })
    return in_maps


def kernel(x, w_attn, b_attn, w_proj, b_proj):
    x = np.asarray(x, dtype=np.float32)
    w_attn = np.asarray(w_attn, dtype=np.float32)
    b_attn = np.asarray(b_attn, dtype=np.float32)
    w_proj = np.asarray(w_proj, dtype=np.float32)
    b_proj = np.asarray(b_proj, dtype=np.float32)

    if np.any(b_attn):
        # Spec guarantees b_attn == 0 (fill: zeros); exact fallback if not.
        return _numpy_reference(x, w_attn, b_attn, w_proj, b_proj)

    from concourse.bass_utils import run_bass_kernel_spmd

    nc = _get_nc()
    in_maps = make_in_maps(x, w_attn)
    for c in range(8):
        g = c % 2
        in_maps[c]["wp"] = np.ascontiguousarray(w_proj[g * GC:(g + 1) * GC, :])

    res = run_bass_kernel_spmd(nc, in_maps, core_ids=list(range(8)))
    y = np.empty((B, S, D), np.float32)
    for b in range(B):
        y[b] = res.results[2 * b]["yT"].T + res.results[2 * b + 1]["yT"].T + b_proj
    return y


def _numpy_reference(x, w_attn, b_attn, w_proj, b_proj):
    qkv = x @ w_attn + b_attn
    q, k, v = np.split(qkv, 3, axis=-1)

    def heads(t):
        return t.reshape(B, S, H, hd).transpose(0, 2, 1, 3)

    q, k, v = heads(q), heads(k), heads(v)
    scores = np.einsum("bhqd,bhkd->bhqk", q, k) / np.sqrt(np.float32(hd))
    causal = np.tril(np.ones((S, S), dtype=bool))[None, None]
    scores = np.where(causal, scores, -1e9)
    scores -= scores.max(axis=-1, keepdims=True)
    attn = np.exp(scores)
    attn /= attn.sum(axis=-1, keepdims=True)
    out = np.einsum("bhqk,bhkd->bhqd", attn, v)
    out = out.transpose(0, 2, 1, 3).reshape(B, S, D)
    return out @ w_proj + b_proj
